# revision 10
# baseline (speedup 1.0000x reference)
"""GATv2-style GAT layer on 8 Trainium2 NeuronCores (Bass/Tile).

Math (per batch b, one batch per core):
  h = x @ W.T                            [N, 256] -> heads [N, 8, 32]
  s_src[n,h] = sum_d h[n,h,d]*a_src[h,d] = x @ Wsrc   (Wsrc folded on host)
  s_dst[n,h] = x @ Wdst
  e[i,j,h]  = leakyrelu_0.2(s_src[i,h] + s_dst[j,h])  masked by adj[i,j]
  alpha     = softmax_j(e);  out[i] = elu(sum_j alpha[i,j,h] h[j,h,:])

Device layout: scores/softmax tensors are [j(partition), i(free)] so the
aggregation matmul contracts j on the partition axis with no transposes of
the big tensors.  Masking is additive before the leaky-relu:
  e' = s_src[i] + s_dst[j] + BIG*(adjT[j,i]-1)
built with one DVE scalar_tensor_tensor per (j-chunk, head).  BIG*adjT is
produced by PE matmuls using adj in its *native* [i,j] layout as the
stationary operand against BIG*I — no explicit transpose of adj needed.
exp() is the single ACT pass; numerator and denominator of the softmax are
both obtained from one PE matmul per tile via an appended ones-column in
h_aug, so no separate row-sum pass exists.  Division by the denominator
happens after aggregation on [N, 264] instead of [N, N, 8].
"""

import numpy as np
from contextlib import ExitStack

import concourse.bass as bass
import concourse.bacc as bacc
import concourse.tile as tile
from concourse import mybir
from concourse.bass_utils import run_bass_kernel_spmd

B, N, IN_DIM, OUT_DIM, H, D = 8, 1024, 256, 256, 8, 32
P = 128
NCH = N // P  # 8 chunks of 128 nodes
F32 = mybir.dt.float32
F16 = mybir.dt.float16
I32 = mybir.dt.int32
ADD = mybir.AluOpType.add
MULT = mybir.AluOpType.mult
MAX = mybir.AluOpType.max
MIN = mybir.AluOpType.min
BIG = 16384.0  # exact in fp16; 0.2*(-BIG) << -87 so exp() underflows to 0
LRELU_HEADS_ON_ACT = 0  # heads whose leaky-relu runs on ScalarE (load balance)


def build_program():
    nc = bacc.Bacc()
    x_t = nc.dram_tensor("x", [N, IN_DIM], F32, kind="ExternalInput")
    adj_t = nc.dram_tensor("adj", [N, N], I32, kind="ExternalInput")
    wt_t = nc.dram_tensor("wt", [IN_DIM, OUT_DIM], F32, kind="ExternalInput")
    wsd_t = nc.dram_tensor("wsd", [IN_DIM, 2 * H], F32, kind="ExternalInput")
    out_t = nc.dram_tensor("out", [N, OUT_DIM], F32, kind="ExternalOutput")
    ident_t = nc.inline_tensor(np.eye(P, dtype=np.float32), "ident")
    bigi_t = nc.inline_tensor((BIG * np.eye(P)).astype(np.float16), "bigi")

    with tile.TileContext(nc) as tc, ExitStack() as ctx:
        consts = ctx.enter_context(tc.tile_pool(name="consts", bufs=1))
        persist = ctx.enter_context(tc.tile_pool(name="persist", bufs=1))
        dram = ctx.enter_context(tc.tile_pool(name="dram", bufs=1, space="DRAM"))

        ident = consts.tile([P, P], F32, tag="ident", name="ident")
        nc.sync.dma_start(out=ident, in_=ident_t[:, :])
        negbig = consts.tile([P, 1], F32, tag="negbig", name="negbig")
        nc.vector.memset(negbig, -BIG)
        bigi = consts.tile([P, P], F16, tag="bigi", name="bigi")
        nc.sync.dma_start(out=bigi, in_=bigi_t[:, :])
        wt_sb = []
        wsd_sb = []
        for k in range(2):
            w = consts.tile([P, OUT_DIM], F32, tag=f"wt{k}", name=f"wt{k}")
            nc.sync.dma_start(out=w, in_=wt_t[k * P : (k + 1) * P, :])
            wt_sb.append(w)
            w2 = consts.tile([P, 2 * H], F32, tag=f"wsd{k}", name=f"wsd{k}")
            nc.sync.dma_start(out=w2, in_=wsd_t[k * P : (k + 1) * P, :])
            wsd_sb.append(w2)

        # ---- persistent SBUF arrays ----
        h_aug = [persist.tile([P, H, D + 1], F16, tag=f"haug{c}", name=f"haug{c}") for c in range(NCH)]
        s_sb = [persist.tile([P, 2 * H], F32, tag=f"s{c}", name=f"s{c}") for c in range(NCH)]
        bcast = [persist.tile([P, N], F16, tag=f"bc{h}", name=f"bc{h}") for h in range(H)]
        maskb = [persist.tile([P, N], F16, tag=f"mb{c}", name=f"mb{c}") for c in range(NCH)]

        # ---- stage 1: x -> xT (PE transpose), h & s matmuls ----
        with tc.tile_pool(name="xstage", bufs=2) as xpool, tc.tile_pool(
            name="xt_ps", bufs=1, space="PSUM"
        ) as xt_ps:
            xT_ps = [xt_ps.tile([P, N], F32, tag=f"xtp{k}", name=f"xtp{k}") for k in range(2)]
            x_sb = []
            for c in range(NCH):
                xc = xpool.tile([P, IN_DIM], F32, tag=f"x{c}", name=f"x{c}")
                nc.sync.dma_start(out=xc, in_=x_t[c * P : (c + 1) * P, :])
                x_sb.append(xc)
                for k in range(2):
                    nc.tensor.transpose(
                        xT_ps[k][:, c * P : (c + 1) * P],
                        xc[:, k * P : (k + 1) * P],
                        ident,
                    )
            xT = [persist.tile([P, N], F32, tag=f"xt{k}", name=f"xt{k}") for k in range(2)]
            for k in range(2):
                nc.scalar.copy(out=xT[k], in_=xT_ps[k])

        with tc.tile_pool(name="hs_ps", bufs=4, space="PSUM") as hs_ps:
            for c in range(NCH):
                ph = hs_ps.tile([P, OUT_DIM], F32, tag="ph", name="ph")
                ps = hs_ps.tile([P, 2 * H], F32, tag="ps", name="ps")
                for k in range(2):
                    lhsT = xT[k][:, c * P : (c + 1) * P]
                    nc.tensor.matmul(ph, lhsT, wt_sb[k], start=(k == 0), stop=(k == 1))
                    nc.tensor.matmul(ps, lhsT, wsd_sb[k], start=(k == 0), stop=(k == 1))
                nc.vector.memset(h_aug[c], 1.0)
                nc.vector.tensor_copy(
                    out=h_aug[c][:, :, 0:D],
                    in_=ph.rearrange("p (h d) -> p h d", h=H),
                )
                nc.vector.tensor_copy(out=s_sb[c], in_=ps)

        # ---- stage 2: s_src rows -> DRAM bounce -> partition-broadcast ----
        with tc.tile_pool(name="st_ps", bufs=1, space="PSUM") as st_ps:
            sT_ps = st_ps.tile([2 * H, N], F32, tag="stp", name="stp")
            for c in range(NCH):
                nc.tensor.transpose(
                    sT_ps[:, c * P : (c + 1) * P], s_sb[c], ident
                )
            srcT = dram.tile([H, N], F16, tag="srcT", name="srcT")
            srcT_sb = persist.tile([H, N], F16, tag="srcT_sb", name="srcT_sb")
            nc.scalar.copy(out=srcT_sb, in_=sT_ps[0:H, :])
            nc.sync.dma_start(out=srcT, in_=srcT_sb)
            for h in range(H):
                row = srcT[h : h + 1, :]
                rep = bass.AP(
                    tensor=row.tensor, offset=row.offset, ap=[[0, P], row.ap[-1]]
                )
                nc.sync.dma_start(out=bcast[h], in_=rep)

        # ---- stage 3: adj -> fp16, BIG*adjT via PE, maskbias = BIG*(adjT-1) ----
        with tc.tile_pool(name="adj_pool", bufs=1) as adj_pool, tc.tile_pool(
            name="m_ps", bufs=2, space="PSUM"
        ) as m_ps:
            adj16 = []
            for c in range(NCH):
                ai = adj_pool.tile([P, N], I32, tag=f"ai{c}", name=f"ai{c}")
                nc.sync.dma_start(out=ai, in_=adj_t[c * P : (c + 1) * P, :])
                af = adj_pool.tile([P, N], F16, tag=f"af{c}", name=f"af{c}")
                nc.gpsimd.tensor_copy(out=af, in_=ai)
                adj16.append(af)
            for jc in range(NCH):
                pm = m_ps.tile([P, N], F32, tag="pm", name="pm")
                for ic in range(NCH):
                    nc.tensor.matmul(
                        pm[:, ic * P : (ic + 1) * P],
                        adj16[ic][:, jc * P : (jc + 1) * P],
                        bigi,
                        start=True,
                        stop=True,
                    )
                nc.scalar.activation(
                    out=maskb[jc],
                    in_=pm,
                    func=mybir.ActivationFunctionType.Identity,
                    bias=negbig[:, 0:1],
                    scale=1.0,
                )

        # ---- main loop: scores -> exp -> masked aggregation ----
        with tc.tile_pool(name="e_pool", bufs=2) as e_pool, tc.tile_pool(
            name="t_pool", bufs=2
        ) as t_pool, tc.tile_pool(name="u_ps", bufs=1, space="PSUM") as u_ps:
            U = [u_ps.tile([P, H, D + 1], F32, tag=f"u{ic}", name=f"u{ic}") for ic in range(NCH)]
            for jc in range(NCH):
                e_t = e_pool.tile([P, H, N], F16, tag="e", name="e")
                t_t = t_pool.tile([P, H, N], F16, tag="t", name="t")
                for h in range(H):
                    nc.vector.scalar_tensor_tensor(
                        out=e_t[:, h, :],
                        in0=bcast[h],
                        scalar=s_sb[jc][:, H + h : H + h + 1],
                        in1=maskb[jc],
                        op0=ADD,
                        op1=ADD,
                    )
                    if h < H - LRELU_HEADS_ON_ACT:
                        nc.vector.scalar_tensor_tensor(
                            out=t_t[:, h, :],
                            in0=e_t[:, h, :],
                            scalar=0.2,
                            in1=e_t[:, h, :],
                            op0=MULT,
                            op1=MAX,
                        )
                    else:
                        nc.scalar.activation(
                            out=t_t[:, h, :],
                            in_=e_t[:, h, :],
                            func=mybir.ActivationFunctionType.Lrelu,
                            alpha=0.2,
                        )
                for g in range(4):
                    nc.scalar.activation(
                        out=t_t[:, 2 * g : 2 * g + 2, :],
                        in_=t_t[:, 2 * g : 2 * g + 2, :],
                        func=mybir.ActivationFunctionType.Exp,
                    )
                for h in range(H):
                    for ic in range(NCH):
                        nc.tensor.matmul(
                            U[ic][:, h, :],
                            t_t[:, h, ic * P : (ic + 1) * P],
                            h_aug[jc][:, h, :],
                            start=(jc == 0 and h == 0),
                            stop=(jc == NCH - 1 and h == H - 1),
                        )

            # ---- normalize + elu + store ----
            with tc.tile_pool(name="fin", bufs=2) as fin:
                for ic in range(NCH):
                    rz = fin.tile([P, H], F32, tag="rz", name="rz")
                    nc.vector.reciprocal(out=rz, in_=U[ic][:, :, D])
                    hn = fin.tile([P, H, D], F32, tag="hn", name="hn")
                    for h in range(H):
                        nc.vector.tensor_scalar(
                            out=hn[:, h, :],
                            in0=U[ic][:, h, 0:D],
                            scalar1=rz[:, h : h + 1],
                            scalar2=None,
                            op0=MULT,
                        )
                    hm = fin.tile([P, H * D], F32, tag="hm", name="hm")
                    nc.vector.tensor_scalar(
                        out=hm, in0=hn, scalar1=0.0, scalar2=None, op0=MIN
                    )
                    he = fin.tile([P, H * D], F32, tag="he", name="he")
                    nc.scalar.activation(
                        out=he, in_=hm, func=mybir.ActivationFunctionType.Exp
                    )
                    hr = fin.tile([P, H * D], F32, tag="hr", name="hr")
                    nc.vector.tensor_scalar(
                        out=hr, in0=hn, scalar1=0.0, scalar2=None, op0=MAX
                    )
                    ho = fin.tile([P, H * D], F32, tag="ho", name="ho")
                    nc.vector.scalar_tensor_tensor(
                        out=ho, in0=he, scalar=-1.0, in1=hr, op0=ADD, op1=ADD
                    )
                    nc.sync.dma_start(
                        out=out_t[ic * P : (ic + 1) * P, :], in_=ho
                    )
    nc.compile()
    return nc


def _fold_weights(W, a):
    # s_src[n,h] = sum_d (x @ W.T)[n, h*D+d] * a_src[h,d] = x @ Wsrc
    Wh = W.reshape(H, D, IN_DIM)  # [h, d, in]
    a_src = a[0, :, :D]  # [H, D]
    a_dst = a[0, :, D:]
    Wsrc = np.einsum("hdi,hd->ih", Wh, a_src)  # [in, H]
    Wdst = np.einsum("hdi,hd->ih", Wh, a_dst)
    wsd = np.concatenate([Wsrc, Wdst], axis=1).astype(np.float32)  # [in, 2H]
    wt = np.ascontiguousarray(W.T).astype(np.float32)  # [in, out]
    return wt, wsd


_NC_CACHE = {}


def _get_program():
    if "nc" not in _NC_CACHE:
        _NC_CACHE["nc"] = build_program()
    return _NC_CACHE["nc"]


def kernel(x, adj, W, a):
    x = np.asarray(x, dtype=np.float32)
    adj = np.ascontiguousarray(np.asarray(adj, dtype=np.int32))
    wt, wsd = _fold_weights(np.asarray(W, dtype=np.float32), np.asarray(a, np.float32))
    nc = _get_program()
    in_maps = [
        {"x": np.ascontiguousarray(x[b]), "adj": adj[b], "wt": wt, "wsd": wsd}
        for b in range(B)
    ]
    res = run_bass_kernel_spmd(nc, in_maps, list(range(B)))
    return np.stack([res.results[b]["out"] for b in range(B)]).astype(np.float32)


if __name__ == "__main__":
    nc = build_program()
    print("build ok:", len(nc.m.functions[0].blocks), "blocks")


# revision 14
# speedup vs baseline: 111.8852x; 111.8852x over previous
"""GATv2-style GAT layer on 8 Trainium2 NeuronCores (Bass/Tile).

Math (per batch b, one batch per core):
  h = x @ W.T                            [N, 256] -> heads [N, 8, 32]
  s_src[n,h] = sum_d h[n,h,d]*a_src[h,d] = x @ Wsrc   (Wsrc folded on host)
  s_dst[n,h] = x @ Wdst
  e[i,j,h]  = leakyrelu_0.2(s_src[i,h] + s_dst[j,h])  masked by adj[i,j]
  alpha     = softmax_j(e);  out[i] = elu(sum_j alpha[i,j,h] h[j,h,:])

Device layout: scores/softmax tensors are [j(partition), i(free)] so the
aggregation matmul contracts j on the partition axis with no transposes of
the big tensors.  Masking is additive before the leaky-relu:
  e' = s_src[i] + s_dst[j] + BIG*(adjT[j,i]-1)
built with one DVE scalar_tensor_tensor per (j-chunk, head).  BIG*adjT is
produced by PE matmuls using adj in its *native* [i,j] layout as the
stationary operand against BIG*I — no explicit transpose of adj needed.
exp() is the single ACT pass; numerator and denominator of the softmax are
both obtained from one PE matmul per tile via an appended ones-column in
h_aug, so no separate row-sum pass exists.  Division by the denominator
happens after aggregation on [N, 264] instead of [N, N, 8].
"""

import numpy as np
from contextlib import ExitStack

import concourse.bass as bass
import concourse.bacc as bacc
import concourse.tile as tile
from concourse import mybir
from concourse.bass_utils import run_bass_kernel_spmd

B, N, IN_DIM, OUT_DIM, H, D = 8, 1024, 256, 256, 8, 32
P = 128
NCH = N // P  # 8 chunks of 128 nodes
F32 = mybir.dt.float32
F16 = mybir.dt.float16
I32 = mybir.dt.int32
ADD = mybir.AluOpType.add
MULT = mybir.AluOpType.mult
MAX = mybir.AluOpType.max
MIN = mybir.AluOpType.min
BIG = 16384.0  # exact in fp16; 0.2*(-BIG) << -87 so exp() underflows to 0
LRELU_HEADS_ON_ACT = 0  # heads whose leaky-relu runs on ScalarE (load balance)


def build_program(iters=1):
    nc = bacc.Bacc()
    x_t = nc.dram_tensor("x", [N, IN_DIM], F32, kind="ExternalInput")
    adj_t = nc.dram_tensor("adj", [N, N], I32, kind="ExternalInput")
    wt_t = nc.dram_tensor("wt", [IN_DIM, OUT_DIM], F32, kind="ExternalInput")
    wsd_t = nc.dram_tensor("wsd", [IN_DIM, 2 * H], F32, kind="ExternalInput")
    out_t = nc.dram_tensor("out", [N, OUT_DIM], F32, kind="ExternalOutput")
    ident_t = nc.inline_tensor(np.eye(P, dtype=np.float32), "ident")
    bigi_t = nc.inline_tensor((BIG * np.eye(P)).astype(np.float16), "bigi")

    with tile.TileContext(nc) as tc, ExitStack() as ctx:
        consts = ctx.enter_context(tc.tile_pool(name="consts", bufs=1))
        persist = ctx.enter_context(tc.tile_pool(name="persist", bufs=1))
        dram = ctx.enter_context(tc.tile_pool(name="dram", bufs=1, space="DRAM"))

        ident = consts.tile([P, P], F32, tag="ident", name="ident")
        nc.sync.dma_start(out=ident, in_=ident_t[:, :])
        negbig = consts.tile([P, 1], F32, tag="negbig", name="negbig")
        nc.vector.memset(negbig, -BIG)
        bigi = consts.tile([P, P], F16, tag="bigi", name="bigi")
        nc.sync.dma_start(out=bigi, in_=bigi_t[:, :])
        wt_sb = []
        wsd_sb = []
        for k in range(2):
            w = consts.tile([P, OUT_DIM], F32, tag=f"wt{k}", name=f"wt{k}")
            nc.sync.dma_start(out=w, in_=wt_t[k * P : (k + 1) * P, :])
            wt_sb.append(w)
            w2 = consts.tile([P, 2 * H], F32, tag=f"wsd{k}", name=f"wsd{k}")
            nc.sync.dma_start(out=w2, in_=wsd_t[k * P : (k + 1) * P, :])
            wsd_sb.append(w2)

        # ---- persistent SBUF arrays ----
        h_aug = [persist.tile([P, H, D + 1], F16, tag=f"haug{c}", name=f"haug{c}") for c in range(NCH)]
        s_sb = [persist.tile([P, 2 * H], F32, tag=f"s{c}", name=f"s{c}") for c in range(NCH)]
        bcast = [persist.tile([P, N], F16, tag=f"bc{h}", name=f"bc{h}") for h in range(H)]
        maskb = [persist.tile([P, N], F16, tag=f"mb{c}", name=f"mb{c}") for c in range(NCH)]

        for _it in range(iters):
            one_pass(nc, tc, x_t, adj_t, out_t, ident_t, bigi_t,
                     consts if _it == 0 else None,
                     persist, dram, h_aug, s_sb, bcast, maskb,
                     wt_sb, wsd_sb, ident, bigi, negbig)
    nc.compile()
    return nc


def one_pass(nc, tc, x_t, adj_t, out_t, ident_t, bigi_t, consts,
             persist, dram, h_aug, s_sb, bcast, maskb,
             wt_sb, wsd_sb, ident, bigi, negbig):
    if True:
        # ---- stage 1: x -> xT (PE transpose), h & s matmuls ----
        with tc.tile_pool(name="xstage", bufs=2) as xpool, tc.tile_pool(
            name="xt_ps", bufs=1, space="PSUM"
        ) as xt_ps:
            xT_ps = [xt_ps.tile([P, N], F32, tag=f"xtp{k}", name=f"xtp{k}") for k in range(2)]
            x_sb = []
            for c in range(NCH):
                xc = xpool.tile([P, IN_DIM], F32, tag=f"x{c}", name=f"x{c}")
                nc.sync.dma_start(out=xc, in_=x_t[c * P : (c + 1) * P, :])
                x_sb.append(xc)
                for k in range(2):
                    nc.tensor.transpose(
                        xT_ps[k][:, c * P : (c + 1) * P],
                        xc[:, k * P : (k + 1) * P],
                        ident,
                    )
            xT = [persist.tile([P, N], F32, tag=f"xt{k}", name=f"xt{k}") for k in range(2)]
            for k in range(2):
                nc.scalar.copy(out=xT[k], in_=xT_ps[k])

        with tc.tile_pool(name="hs_ps", bufs=4, space="PSUM") as hs_ps:
            for c in range(NCH):
                ph = hs_ps.tile([P, OUT_DIM], F32, tag="ph", name="ph")
                ps = hs_ps.tile([P, 2 * H], F32, tag="ps", name="ps")
                for k in range(2):
                    lhsT = xT[k][:, c * P : (c + 1) * P]
                    nc.tensor.matmul(ph, lhsT, wt_sb[k], start=(k == 0), stop=(k == 1))
                    nc.tensor.matmul(ps, lhsT, wsd_sb[k], start=(k == 0), stop=(k == 1))
                nc.vector.memset(h_aug[c], 1.0)
                nc.vector.tensor_copy(
                    out=h_aug[c][:, :, 0:D],
                    in_=ph.rearrange("p (h d) -> p h d", h=H),
                )
                nc.vector.tensor_copy(out=s_sb[c], in_=ps)

        # ---- stage 2: s_src rows -> DRAM bounce -> partition-broadcast ----
        with tc.tile_pool(name="st_ps", bufs=1, space="PSUM") as st_ps:
            sT_ps = st_ps.tile([2 * H, N], F32, tag="stp", name="stp")
            for c in range(NCH):
                nc.tensor.transpose(
                    sT_ps[:, c * P : (c + 1) * P], s_sb[c], ident
                )
            srcT = dram.tile([H, N], F16, tag="srcT", name="srcT")
            srcT_sb = persist.tile([H, N], F16, tag="srcT_sb", name="srcT_sb")
            nc.scalar.copy(out=srcT_sb, in_=sT_ps[0:H, :])
            nc.sync.dma_start(out=srcT, in_=srcT_sb)
            for h in range(H):
                row = srcT[h : h + 1, :]
                rep = bass.AP(
                    tensor=row.tensor, offset=row.offset, ap=[[0, P], row.ap[-1]]
                )
                nc.sync.dma_start(out=bcast[h], in_=rep)

        # ---- stage 3: adj -> fp16, BIG*adjT via PE, maskbias = BIG*(adjT-1) ----
        with tc.tile_pool(name="adj_pool", bufs=1) as adj_pool, tc.tile_pool(
            name="m_ps", bufs=2, space="PSUM"
        ) as m_ps:
            adj16 = []
            for c in range(NCH):
                ai = adj_pool.tile([P, N], I32, tag=f"ai{c}", name=f"ai{c}")
                nc.sync.dma_start(out=ai, in_=adj_t[c * P : (c + 1) * P, :])
                af = adj_pool.tile([P, N], F16, tag=f"af{c}", name=f"af{c}")
                nc.gpsimd.tensor_copy(out=af, in_=ai)
                adj16.append(af)
            for jc in range(NCH):
                pm = m_ps.tile([P, N], F32, tag="pm", name="pm")
                for ic in range(NCH):
                    nc.tensor.matmul(
                        pm[:, ic * P : (ic + 1) * P],
                        adj16[ic][:, jc * P : (jc + 1) * P],
                        bigi,
                        start=True,
                        stop=True,
                    )
                nc.scalar.activation(
                    out=maskb[jc],
                    in_=pm,
                    func=mybir.ActivationFunctionType.Identity,
                    bias=negbig[:, 0:1],
                    scale=1.0,
                )

        # ---- main loop: scores -> exp -> masked aggregation ----
        with tc.tile_pool(name="e_pool", bufs=2) as e_pool, tc.tile_pool(
            name="t_pool", bufs=2
        ) as t_pool, tc.tile_pool(name="u_ps", bufs=1, space="PSUM") as u_ps:
            U = [u_ps.tile([P, H, D + 1], F32, tag=f"u{ic}", name=f"u{ic}") for ic in range(NCH)]
            for jc in range(NCH):
                e_t = e_pool.tile([P, H, N], F16, tag="e", name="e")
                t_t = t_pool.tile([P, H, N], F16, tag="t", name="t")
                for h in range(H):
                    nc.vector.scalar_tensor_tensor(
                        out=e_t[:, h, :],
                        in0=bcast[h],
                        scalar=s_sb[jc][:, H + h : H + h + 1],
                        in1=maskb[jc],
                        op0=ADD,
                        op1=ADD,
                    )
                    if h < H - LRELU_HEADS_ON_ACT:
                        nc.vector.scalar_tensor_tensor(
                            out=t_t[:, h, :],
                            in0=e_t[:, h, :],
                            scalar=0.2,
                            in1=e_t[:, h, :],
                            op0=MULT,
                            op1=MAX,
                        )
                    else:
                        nc.scalar.activation(
                            out=t_t[:, h, :],
                            in_=e_t[:, h, :],
                            func=mybir.ActivationFunctionType.Lrelu,
                            alpha=0.2,
                        )
                for g in range(4):
                    nc.scalar.activation(
                        out=t_t[:, 2 * g : 2 * g + 2, :],
                        in_=t_t[:, 2 * g : 2 * g + 2, :],
                        func=mybir.ActivationFunctionType.Exp,
                    )
                for h in range(H):
                    for ic in range(NCH):
                        nc.tensor.matmul(
                            U[ic][:, h, :],
                            t_t[:, h, ic * P : (ic + 1) * P],
                            h_aug[jc][:, h, :],
                            start=(jc == 0 and h == 0),
                            stop=(jc == NCH - 1 and h == H - 1),
                        )

            # ---- normalize + elu + store ----
            with tc.tile_pool(name="fin", bufs=2) as fin:
                for ic in range(NCH):
                    rz = fin.tile([P, H], F32, tag="rz", name="rz")
                    nc.vector.reciprocal(out=rz, in_=U[ic][:, :, D])
                    hn = fin.tile([P, H, D], F32, tag="hn", name="hn")
                    for h in range(H):
                        nc.vector.tensor_scalar(
                            out=hn[:, h, :],
                            in0=U[ic][:, h, 0:D],
                            scalar1=rz[:, h : h + 1],
                            scalar2=None,
                            op0=MULT,
                        )
                    hm = fin.tile([P, H * D], F32, tag="hm", name="hm")
                    nc.vector.tensor_scalar(
                        out=hm, in0=hn, scalar1=0.0, scalar2=None, op0=MIN
                    )
                    he = fin.tile([P, H * D], F32, tag="he", name="he")
                    nc.scalar.activation(
                        out=he, in_=hm, func=mybir.ActivationFunctionType.Exp
                    )
                    hr = fin.tile([P, H * D], F32, tag="hr", name="hr")
                    nc.vector.tensor_scalar(
                        out=hr, in0=hn, scalar1=0.0, scalar2=None, op0=MAX
                    )
                    ho = fin.tile([P, H * D], F32, tag="ho", name="ho")
                    nc.vector.scalar_tensor_tensor(
                        out=ho, in0=he, scalar=-1.0, in1=hr, op0=ADD, op1=ADD
                    )
                    nc.sync.dma_start(
                        out=out_t[ic * P : (ic + 1) * P, :], in_=ho
                    )


def _fold_weights(W, a):
    # s_src[n,h] = sum_d (x @ W.T)[n, h*D+d] * a_src[h,d] = x @ Wsrc
    Wh = W.reshape(H, D, IN_DIM)  # [h, d, in]
    a_src = a[0, :, :D]  # [H, D]
    a_dst = a[0, :, D:]
    Wsrc = np.einsum("hdi,hd->ih", Wh, a_src)  # [in, H]
    Wdst = np.einsum("hdi,hd->ih", Wh, a_dst)
    wsd = np.concatenate([Wsrc, Wdst], axis=1).astype(np.float32)  # [in, 2H]
    wt = np.ascontiguousarray(W.T).astype(np.float32)  # [in, out]
    return wt, wsd


_NC_CACHE = {}


def _get_program():
    if "nc" not in _NC_CACHE:
        _NC_CACHE["nc"] = build_program()
    return _NC_CACHE["nc"]


def kernel(x, adj, W, a):
    x = np.asarray(x, dtype=np.float32)
    adj = np.ascontiguousarray(np.asarray(adj, dtype=np.int32))
    wt, wsd = _fold_weights(np.asarray(W, dtype=np.float32), np.asarray(a, np.float32))
    nc = _get_program()
    in_maps = [
        {"x": np.ascontiguousarray(x[b]), "adj": adj[b], "wt": wt, "wsd": wsd}
        for b in range(B)
    ]
    res = run_bass_kernel_spmd(nc, in_maps, list(range(B)))
    return np.stack([res.results[b]["out"] for b in range(B)]).astype(np.float32)


if __name__ == "__main__":
    nc = build_program()
    print("build ok:", len(nc.m.functions[0].blocks), "blocks")


# revision 18
# speedup vs baseline: 240.9542x; 2.1536x over previous
"""GATv2-style GAT layer on 8 Trainium2 NeuronCores (Bass/Tile).

Math (per batch b, one batch per core):
  h = x @ W.T                            [N, 256] -> heads [N, 8, 32]
  s_src[n,h] = sum_d h[n,h,d]*a_src[h,d] = x @ Wsrc   (Wsrc folded on host)
  s_dst[n,h] = x @ Wdst
  e[i,j,h]  = leakyrelu_0.2(s_src[i,h] + s_dst[j,h])  masked by adj[i,j]
  alpha     = softmax_j(e);  out[i] = elu(sum_j alpha[i,j,h] h[j,h,:])

Device layout: scores/softmax tensors are [j(partition), i(free)] so the
aggregation matmul contracts j on the partition axis with no transposes of
the big tensors.  Masking is additive before the leaky-relu:
  e' = s_src[i] + s_dst[j] + BIG*(adjT[j,i]-1)
built with one DVE scalar_tensor_tensor per (j-chunk, head).  BIG*adjT is
produced by PE matmuls using adj in its *native* [i,j] layout as the
stationary operand against BIG*I — no explicit transpose of adj needed.
exp() is the single ACT pass; numerator and denominator of the softmax are
both obtained from one PE matmul per tile via an appended ones-column in
h_aug, so no separate row-sum pass exists.  Division by the denominator
happens after aggregation on [N, 264] instead of [N, N, 8].
"""

import numpy as np
from contextlib import ExitStack

import concourse.bass as bass
import concourse.bacc as bacc
import concourse.tile as tile
from concourse import mybir
from concourse.bass_utils import run_bass_kernel_spmd

B, N, IN_DIM, OUT_DIM, H, D = 8, 1024, 256, 256, 8, 32
P = 128
NCH = N // P  # 8 chunks of 128 nodes
F32 = mybir.dt.float32
F16 = mybir.dt.float16
I32 = mybir.dt.int32
ADD = mybir.AluOpType.add
MULT = mybir.AluOpType.mult
MAX = mybir.AluOpType.max
MIN = mybir.AluOpType.min
BIG = 16384.0  # exact in fp16; 0.2*(-BIG) << -87 so exp() underflows to 0
LRELU_HEADS_ON_ACT = 0  # heads whose leaky-relu runs on ScalarE (load balance)

# Bisection switches (timing experiments only — numerics are garbage when set)
SKIP = set()  # subset of {"dve_scores", "exp", "agg", "maskmm", "bcast_setup"}


def build_program(iters=1):
    nc = bacc.Bacc()
    x_t = nc.dram_tensor("x", [N, IN_DIM], F32, kind="ExternalInput")
    adj_t = nc.dram_tensor("adj", [N, N], I32, kind="ExternalInput")
    wt_t = nc.dram_tensor("wt", [IN_DIM, OUT_DIM], F32, kind="ExternalInput")
    wsd_t = nc.dram_tensor("wsd", [IN_DIM, 2 * H], F32, kind="ExternalInput")
    out_t = nc.dram_tensor("out", [N, OUT_DIM], F32, kind="ExternalOutput")
    ident_t = nc.inline_tensor(np.eye(P, dtype=np.float32), "ident")
    bigi_t = nc.inline_tensor((BIG * np.eye(P)).astype(np.float16), "bigi")

    with tile.TileContext(nc) as tc, ExitStack() as ctx:
        consts = ctx.enter_context(tc.tile_pool(name="consts", bufs=1))
        persist = ctx.enter_context(tc.tile_pool(name="persist", bufs=1))
        dram = ctx.enter_context(tc.tile_pool(name="dram", bufs=1, space="DRAM"))

        ident = consts.tile([P, P], F32, tag="ident", name="ident")
        nc.sync.dma_start(out=ident, in_=ident_t[:, :])
        negbig = consts.tile([P, 1], F32, tag="negbig", name="negbig")
        nc.vector.memset(negbig, -BIG)
        bigi = consts.tile([P, P], F16, tag="bigi", name="bigi")
        nc.sync.dma_start(out=bigi, in_=bigi_t[:, :])
        wt_sb = []
        wsd_sb = []
        for k in range(2):
            w = consts.tile([P, OUT_DIM], F32, tag=f"wt{k}", name=f"wt{k}")
            nc.sync.dma_start(out=w, in_=wt_t[k * P : (k + 1) * P, :])
            wt_sb.append(w)
            w2 = consts.tile([P, 2 * H], F32, tag=f"wsd{k}", name=f"wsd{k}")
            nc.sync.dma_start(out=w2, in_=wsd_t[k * P : (k + 1) * P, :])
            wsd_sb.append(w2)

        # ---- persistent SBUF arrays ----
        h_aug = [persist.tile([P, H, D + 1], F16, tag=f"haug{c}", name=f"haug{c}") for c in range(NCH)]
        s_sb = [persist.tile([P, 2 * H], F32, tag=f"s{c}", name=f"s{c}") for c in range(NCH)]
        bcast = [persist.tile([P, N], F16, tag=f"bc{h}", name=f"bc{h}") for h in range(H)]
        maskb = [persist.tile([P, N], F16, tag=f"mb{c}", name=f"mb{c}") for c in range(NCH)]

        for _it in range(iters):
            one_pass(nc, tc, x_t, adj_t, out_t, ident_t, bigi_t,
                     consts if _it == 0 else None,
                     persist, dram, h_aug, s_sb, bcast, maskb,
                     wt_sb, wsd_sb, ident, bigi, negbig)
    nc.compile()
    return nc


def one_pass(nc, tc, x_t, adj_t, out_t, ident_t, bigi_t, consts,
             persist, dram, h_aug, s_sb, bcast, maskb,
             wt_sb, wsd_sb, ident, bigi, negbig):
    if True:
        # ---- stage 1: x -> xT (PE transpose), h & s matmuls ----
        with tc.tile_pool(name="xstage", bufs=2) as xpool, tc.tile_pool(
            name="xt_ps", bufs=1, space="PSUM"
        ) as xt_ps:
            xT_ps = [xt_ps.tile([P, N], F32, tag=f"xtp{k}", name=f"xtp{k}") for k in range(2)]
            x_sb = []
            for c in range(NCH):
                xc = xpool.tile([P, IN_DIM], F32, tag=f"x{c}", name=f"x{c}")
                nc.sync.dma_start(out=xc, in_=x_t[c * P : (c + 1) * P, :])
                x_sb.append(xc)
                for k in range(2):
                    nc.tensor.transpose(
                        xT_ps[k][:, c * P : (c + 1) * P],
                        xc[:, k * P : (k + 1) * P],
                        ident,
                    )
            xT = [persist.tile([P, N], F32, tag=f"xt{k}", name=f"xt{k}") for k in range(2)]
            for k in range(2):
                nc.scalar.copy(out=xT[k], in_=xT_ps[k])

        with tc.tile_pool(name="hs_ps", bufs=4, space="PSUM") as hs_ps:
            for c in range(NCH):
                ph = hs_ps.tile([P, OUT_DIM], F32, tag="ph", name="ph")
                ps = hs_ps.tile([P, 2 * H], F32, tag="ps", name="ps")
                for k in range(2):
                    lhsT = xT[k][:, c * P : (c + 1) * P]
                    nc.tensor.matmul(ph, lhsT, wt_sb[k], start=(k == 0), stop=(k == 1))
                    nc.tensor.matmul(ps, lhsT, wsd_sb[k], start=(k == 0), stop=(k == 1))
                nc.vector.memset(h_aug[c], 1.0)
                nc.vector.tensor_copy(
                    out=h_aug[c][:, :, 0:D],
                    in_=ph.rearrange("p (h d) -> p h d", h=H),
                )
                nc.vector.tensor_copy(out=s_sb[c], in_=ps)

        # ---- stage 2: s_src rows -> DRAM bounce -> partition-broadcast ----
        with tc.tile_pool(name="st_ps", bufs=1, space="PSUM") as st_ps:
            sT_ps = st_ps.tile([2 * H, N], F32, tag="stp", name="stp")
            for c in range(NCH):
                nc.tensor.transpose(
                    sT_ps[:, c * P : (c + 1) * P], s_sb[c], ident
                )
            srcT = dram.tile([H, N], F16, tag="srcT", name="srcT")
            srcT_sb = persist.tile([H, N], F16, tag="srcT_sb", name="srcT_sb")
            nc.scalar.copy(out=srcT_sb, in_=sT_ps[0:H, :])
            if "bcast_setup" not in SKIP:
                nc.sync.dma_start(out=srcT, in_=srcT_sb)
                for h in range(H):
                    row = srcT[h : h + 1, :]
                    rep = bass.AP(
                        tensor=row.tensor, offset=row.offset, ap=[[0, P], row.ap[-1]]
                    )
                    nc.sync.dma_start(out=bcast[h], in_=rep)

        # ---- stage 3: adj -> fp16, BIG*adjT via PE, maskbias = BIG*(adjT-1) ----
        with tc.tile_pool(name="adj_pool", bufs=1) as adj_pool, tc.tile_pool(
            name="m_ps", bufs=2, space="PSUM"
        ) as m_ps:
            adj16 = []
            for c in range(NCH):
                ai = adj_pool.tile([P, N], I32, tag=f"ai{c}", name=f"ai{c}")
                nc.sync.dma_start(out=ai, in_=adj_t[c * P : (c + 1) * P, :])
                af = adj_pool.tile([P, N], F16, tag=f"af{c}", name=f"af{c}")
                nc.gpsimd.tensor_copy(out=af, in_=ai)
                adj16.append(af)
            for jc in range(NCH if "maskmm" not in SKIP else 0):
                pm = m_ps.tile([P, N], F32, tag="pm", name="pm")
                for ic in range(NCH):
                    nc.tensor.matmul(
                        pm[:, ic * P : (ic + 1) * P],
                        adj16[ic][:, jc * P : (jc + 1) * P],
                        bigi,
                        start=True,
                        stop=True,
                    )
                nc.scalar.activation(
                    out=maskb[jc],
                    in_=pm,
                    func=mybir.ActivationFunctionType.Identity,
                    bias=negbig[:, 0:1],
                    scale=1.0,
                )

        # ---- main loop: scores -> exp -> masked aggregation ----
        with tc.tile_pool(name="e_pool", bufs=2) as e_pool, tc.tile_pool(
            name="t_pool", bufs=2
        ) as t_pool, tc.tile_pool(name="u_ps", bufs=1, space="PSUM") as u_ps:
            U = [u_ps.tile([P, H, D + 1], F32, tag=f"u{ic}", name=f"u{ic}") for ic in range(NCH)]
            for jc in range(NCH):
                e_t = e_pool.tile([P, H, N], F16, tag="e", name="e")
                t_t = t_pool.tile([P, H, N], F16, tag="t", name="t")
                if "dve_scores" not in SKIP:
                    for h in range(H):
                        nc.vector.scalar_tensor_tensor(
                            out=e_t[:, h, :],
                            in0=bcast[h],
                            scalar=s_sb[jc][:, H + h : H + h + 1],
                            in1=maskb[jc],
                            op0=ADD,
                            op1=ADD,
                        )
                        if h < H - LRELU_HEADS_ON_ACT:
                            nc.vector.scalar_tensor_tensor(
                                out=t_t[:, h, :],
                                in0=e_t[:, h, :],
                                scalar=0.2,
                                in1=e_t[:, h, :],
                                op0=MULT,
                                op1=MAX,
                            )
                        else:
                            nc.scalar.activation(
                                out=t_t[:, h, :],
                                in_=e_t[:, h, :],
                                func=mybir.ActivationFunctionType.Lrelu,
                                alpha=0.2,
                            )
                if "exp" not in SKIP:
                    for g in range(4):
                        nc.scalar.activation(
                            out=t_t[:, 2 * g : 2 * g + 2, :],
                            in_=t_t[:, 2 * g : 2 * g + 2, :],
                            func=mybir.ActivationFunctionType.Exp,
                        )
                if "agg" not in SKIP:
                    for h in range(H):
                        for ic in range(NCH):
                            nc.tensor.matmul(
                                U[ic][:, h, :],
                                t_t[:, h, ic * P : (ic + 1) * P],
                                h_aug[jc][:, h, :],
                                start=(jc == 0 and h == 0),
                                stop=(jc == NCH - 1 and h == H - 1),
                            )

            # ---- normalize + elu + store ----
            with tc.tile_pool(name="fin", bufs=2) as fin:
                for ic in range(NCH):
                    rz = fin.tile([P, H], F32, tag="rz", name="rz")
                    nc.vector.reciprocal(out=rz, in_=U[ic][:, :, D])
                    hn = fin.tile([P, H, D], F32, tag="hn", name="hn")
                    for h in range(H):
                        nc.vector.tensor_scalar(
                            out=hn[:, h, :],
                            in0=U[ic][:, h, 0:D],
                            scalar1=rz[:, h : h + 1],
                            scalar2=None,
                            op0=MULT,
                        )
                    hm = fin.tile([P, H * D], F32, tag="hm", name="hm")
                    nc.vector.tensor_scalar(
                        out=hm, in0=hn, scalar1=0.0, scalar2=None, op0=MIN
                    )
                    he = fin.tile([P, H * D], F32, tag="he", name="he")
                    nc.scalar.activation(
                        out=he, in_=hm, func=mybir.ActivationFunctionType.Exp
                    )
                    hr = fin.tile([P, H * D], F32, tag="hr", name="hr")
                    nc.vector.tensor_scalar(
                        out=hr, in0=hn, scalar1=0.0, scalar2=None, op0=MAX
                    )
                    ho = fin.tile([P, H * D], F32, tag="ho", name="ho")
                    nc.vector.scalar_tensor_tensor(
                        out=ho, in0=he, scalar=-1.0, in1=hr, op0=ADD, op1=ADD
                    )
                    nc.sync.dma_start(
                        out=out_t[ic * P : (ic + 1) * P, :], in_=ho
                    )


def _fold_weights(W, a):
    # s_src[n,h] = sum_d (x @ W.T)[n, h*D+d] * a_src[h,d] = x @ Wsrc
    Wh = W.reshape(H, D, IN_DIM)  # [h, d, in]
    a_src = a[0, :, :D]  # [H, D]
    a_dst = a[0, :, D:]
    Wsrc = np.einsum("hdi,hd->ih", Wh, a_src)  # [in, H]
    Wdst = np.einsum("hdi,hd->ih", Wh, a_dst)
    wsd = np.concatenate([Wsrc, Wdst], axis=1).astype(np.float32)  # [in, 2H]
    wt = np.ascontiguousarray(W.T).astype(np.float32)  # [in, out]
    return wt, wsd


_NC_CACHE = {}


def _get_program():
    if "nc" not in _NC_CACHE:
        _NC_CACHE["nc"] = build_program()
    return _NC_CACHE["nc"]


def kernel(x, adj, W, a):
    x = np.asarray(x, dtype=np.float32)
    adj = np.ascontiguousarray(np.asarray(adj, dtype=np.int32))
    wt, wsd = _fold_weights(np.asarray(W, dtype=np.float32), np.asarray(a, np.float32))
    nc = _get_program()
    in_maps = [
        {"x": np.ascontiguousarray(x[b]), "adj": adj[b], "wt": wt, "wsd": wsd}
        for b in range(B)
    ]
    res = run_bass_kernel_spmd(nc, in_maps, list(range(B)))
    return np.stack([res.results[b]["out"] for b in range(B)]).astype(np.float32)


if __name__ == "__main__":
    nc = build_program()
    print("build ok:", len(nc.m.functions[0].blocks), "blocks")
